# revision 22
# baseline (speedup 1.0000x reference)
"""Multi-head causal attention (B=16, T=512, D=1024, H=16) on 8 TRN2 cores.

Sharding: data-parallel over batch (2 batches per core), weights replicated.
Per-core kernel:
  x -> x^T (PE transpose, fp32r) -> Q^T,K^T (feature-major, bf16 via ACT
  evac with per-partition bias) and V (token-major bf16, ones column per
  head for softmax row sums; bias added by DVE during evac)
  S^T = K_h^T.T @ Q_h^T per head in bf16 (only blocks with q >= k; causal
      mask on diagonal blocks via bf16 ident@maskT matmul in PSUM; the
      kt=1 and kt=3 chunks share one PSUM bank so exp needs 3 ACT ops)
  P^T = exp(S^T/32) on ACT (bf16) -> y'^T = V'_h.T @ P^T with sum row 64
  normalize: DVE recip of sum row, Pool partition_broadcast, DVE multiply
  (deferred two heads so the Pool hop never blocks DVE)
  out = y^T.T @ w_o (bf16, converted on the idle Pool engine) with bias
  added by DVE tensor_add during PSUM evac
Schedule: attention emits S(i+1) before AV(i) and fills the exp latency
with useful PE work per head: x^T(b1) transpose groups then Q(b1)
fo-groups in attention(b0), w_o matmul quads in attention(b1); the last
two head normalizes are covered by the tail chunks. QKV streams weight
k-tiles (9-ring) against open PSUM accumulations; V runs fo-outer from a
resident tile set so evacuations land before attention needs them.
"""

import sys

sys.path.insert(0, "/opt/trn_rl_repo")

import numpy as np

B, T, D = 16, 512, 1024
H = 16
HD = D // H          # 64
NCORES = 8
BL = B // NCORES     # 2 local batches per core
PPART = 128
VW = HD + 1          # 65: head dim + ones column for row sums

_CACHE = {}


def _build_program(reps=1, phases="xqaw"):
    import concourse.bass as bass
    import concourse.tile as tile
    from concourse import bacc, mybir
    from concourse.masks import make_identity

    DT = mybir.dt.float32
    R = mybir.dt.float32r
    BF = mybir.dt.bfloat16
    ACTF = mybir.ActivationFunctionType
    ALU = mybir.AluOpType

    nc = bacc.Bacc("TRN2", target_bir_lowering=False, debug=False,
                   num_devices=NCORES)

    x_d = nc.dram_tensor("x", [BL, T, D], DT, kind="ExternalInput").ap()
    wqkv_d = nc.dram_tensor("w_qkv", [D, 3 * D], DT, kind="ExternalInput").ap()
    bqkv_d = nc.dram_tensor("b_qkv", [3 * D], DT, kind="ExternalInput").ap()
    wo_d = nc.dram_tensor("w_o", [D, D], DT, kind="ExternalInput").ap()
    bo_d = nc.dram_tensor("b_o", [D], DT, kind="ExternalInput").ap()
    out_d = nc.dram_tensor("out", [BL, T, D], DT, kind="ExternalOutput").ap()

    x_f = x_d.flatten_outer_dims()      # [1024, 1024] tokens x features
    out_fs = [out_d.flatten_outer_dims()]
    for r in range(1, reps):
        scr = nc.dram_tensor(f"scratch{r}", [BL, T, D], DT).ap()
        out_fs.append(scr.flatten_outer_dims())

    def f32r(ap):
        return ap.bitcast(R)

    with tile.TileContext(nc) as tc:
        with (
            tc.tile_pool(name="consts", bufs=1) as consts,
            tc.tile_pool(name="y", bufs=1) as y_pool,
            tc.tile_pool(name="xt", bufs=1) as xt_pool,
            tc.tile_pool(name="qkv", bufs=1) as qkv_pool,
            tc.tile_pool(name="w", bufs=8) as w_pool,
            tc.tile_pool(name="pre", bufs=8) as pre_pool,
            tc.tile_pool(name="wo", bufs=8) as wo_pool,
            tc.tile_pool(name="xn", bufs=3) as xn_pool,
            tc.tile_pool(name="pp", bufs=4) as p_pool,
            tc.tile_pool(name="ss", bufs=3) as s_pool,
            tc.tile_pool(name="ob", bufs=3) as o_pool,
        ):
            # ---------------- constants ----------------
            ident_f = consts.tile([PPART, PPART], DT)
            make_identity(nc, ident_f)
            ident = consts.tile([PPART, PPART], R)
            nc.vector.tensor_copy(out=ident, in_=ident_f)

            # transposed causal 0/1 mask for S^T layout: entry (k=i, q=j)
            # is kept (1.0) where j - i >= 0, else zeroed; applied to the
            # exp output on DVE (2-byte SBUF ops run at 4x rate there)
            mask1 = consts.tile([PPART, PPART], DT)
            nc.vector.memset(mask1, 1.0)
            nc.gpsimd.affine_select(
                out=mask1, in_=mask1,
                compare_op=ALU.is_ge, fill=0.0,
                base=0, pattern=[[1, PPART]], channel_multiplier=-1,
            )
            mask1_b = consts.tile([PPART, PPART], BF)
            nc.vector.tensor_copy(out=mask1_b, in_=mask1)

            # qkv bias, feature-major columns: bcol[p, c] = b_qkv[128c + p]
            bcol = consts.tile([PPART, 24], DT)
            # V bias and output bias broadcast to all partitions
            brow = consts.tile([1, D], DT)
            bv_bc = consts.tile([PPART, D], DT)
            bo_bc = consts.tile([PPART, D], DT)

            def load_biases():
                nc.sync.dma_start(
                    out=bcol, in_=bqkv_d.rearrange("(c p) -> p c", p=PPART))
                nc.sync.dma_start(
                    out=brow,
                    in_=bqkv_d[2 * D:3 * D].rearrange("(a f) -> a f", a=1))
                nc.gpsimd.partition_broadcast(bv_bc, brow, channels=PPART)
                nc.sync.dma_start(
                    out=brow, in_=bo_d.rearrange("(a f) -> a f", a=1))
                nc.gpsimd.partition_broadcast(bo_bc, brow, channels=PPART)

            y_t = y_pool.tile([PPART, 8, BL * T], BF)  # [128, 8, 1024]

            def start_xn_dma(b, to, split=False):
                xn = xn_pool.tile([PPART, D], R)
                src = x_f[T * b + 128 * to:T * b + 128 * (to + 1), :]
                if split:
                    for c in range(4):
                        nc.sync.dma_start(
                            out=xn[:, 256 * c:256 * (c + 1)],
                            in_=f32r(src[:, 256 * c:256 * (c + 1)]))
                else:
                    nc.sync.dma_start(out=xn, in_=f32r(src))
                return xn

            def transpose_fg(x_t, xn, to, fg, trps):
                # one group: 4 transposes + 1 copy, via a DT-tagged bank
                pst_d = trps.tile([PPART, 4 * PPART], DT, tag="tr")
                pst = pst_d.bitcast(R).rearrange("p (f q) -> p f q", f=4)
                for fi in range(4):
                    fo = 4 * fg + fi
                    nc.tensor.transpose(
                        pst[:, fi, :], xn[:, 128 * fo:128 * (fo + 1)], ident)
                nc.vector.tensor_copy(
                    out=x_t[:, 4 * fg:4 * (fg + 1), 128 * to:128 * (to + 1)],
                    in_=pst)

            def w_dma(src_ap):
                w_sb = w_pool.tile([PPART, 1024], R, tag="w")
                nc.sync.dma_start(out=w_sb, in_=f32r(src_ap))
                return w_sb

            def preload_sec(sec):
                tiles = []
                for ko in range(8):
                    w_sb = pre_pool.tile([PPART, 1024], R, tag="w",
                                         name=f"wpre{sec}_{ko}")
                    nc.sync.dma_start(
                        out=w_sb,
                        in_=f32r(wqkv_d[128 * ko:128 * (ko + 1),
                                        1024 * sec:1024 * (sec + 1)]))
                    tiles.append(w_sb)
                return tiles

            def alloc_qkv(which):
                t = {}
                if "q" in which:
                    t["q"] = qkv_pool.tile([PPART, 8, T], BF, tag="q", name="q_t")
                if "k" in which:
                    t["k"] = qkv_pool.tile([PPART, 8, T], BF, tag="k", name="k_t")
                if "v" in which:
                    v_t = qkv_pool.tile([PPART, 4, H * VW], BF, tag="v")
                    ones = v_t.rearrange("p t (h c) -> p t h c",
                                         c=VW)[:, :, :, HD:]
                    nc.vector.memset(ones, 1.0)
                    t["v"] = v_t
                return t

            def q_evac(ps, q_t, fo):
                nc.scalar.activation(
                    out=q_t[:, fo, :], in_=ps, func=ACTF.Identity,
                    bias=bcol[:, fo:fo + 1])

            def qkv_sec_stream(x_t, qps, dst, sec, tiles=None):
                # one projection section, ko-outer over 8 open PSUM banks
                psums = [qps.tile([PPART, T], DT, tag="ps", name=f"qkvps{i}")
                         for i in range(8)]
                for ko in range(8):
                    w_sb = (tiles[ko] if tiles is not None else
                            w_dma(wqkv_d[128 * ko:128 * (ko + 1),
                                         1024 * sec:1024 * (sec + 1)]))
                    for fo in range(8):
                        nc.tensor.matmul(
                            psums[fo],
                            lhsT=w_sb[:, 128 * fo:128 * (fo + 1)],
                            rhs=x_t[:, ko, :],
                            start=(ko == 0), stop=(ko == 7))
                for fo in range(8):
                    nc.scalar.activation(
                        out=dst[:, fo, :], in_=psums[fo],
                        func=ACTF.Identity,
                        bias=bcol[:, 8 * sec + fo:8 * sec + fo + 1])

            def v_evac(ps, v_t, to, nh):
                vv = v_t[:, to, 8 * VW * nh:8 * VW * (nh + 1)]
                vv = vv.rearrange("p (h c) -> p h c", c=VW)[:, :, :HD]
                nc.vector.tensor_add(
                    out=vv, in0=ps, in1=bv_bc[:, 512 * nh:512 * (nh + 1)])

            def qkv_v_sec(b, x_t, qps, v_t, resident):
                if resident:
                    # fo-outer from a resident tile set: each PSUM stops
                    # and evacuates as soon as its 8 accumulations finish
                    wv = [w_dma(wqkv_d[128 * ko:128 * (ko + 1), 2048:3072])
                          for ko in range(8)]
                    for nh in range(2):   # nh-major: heads 0-7 evac first
                        for to in range(4):
                            ps = qps.tile([PPART, T], DT, tag="ps")
                            for ko in range(8):
                                nc.tensor.matmul(
                                    ps,
                                    lhsT=x_t[:, ko, 128 * to:128 * (to + 1)],
                                    rhs=wv[ko][:, 512 * nh:512 * (nh + 1)],
                                    start=(ko == 0), stop=(ko == 7))
                            v_evac(ps, v_t, to, nh)
                else:
                    # ko-outer: streams tiles as the DMAs land
                    psums = [qps.tile([PPART, T], DT, tag="ps",
                                      name=f"vps{i}") for i in range(8)]
                    for ko in range(8):
                        w_sb = w_dma(wqkv_d[128 * ko:128 * (ko + 1),
                                            2048:3072])
                        for to in range(4):
                            for nh in range(2):
                                nc.tensor.matmul(
                                    psums[2 * to + nh],
                                    lhsT=x_t[:, ko, 128 * to:128 * (to + 1)],
                                    rhs=w_sb[:, 512 * nh:512 * (nh + 1)],
                                    start=(ko == 0), stop=(ko == 7))
                    for nh in range(2):
                        for to in range(4):
                            v_evac(psums[2 * to + nh], v_t, to, nh)

            wo_tiles = {}

            def load_wo():
                # stage fp32 through the (now dead) Q-preload tiles, then
                # convert to bf16 on the otherwise-idle Pool engine
                stages = []
                for ko in range(8):
                    stage = pre_pool.tile([PPART, 1024], R, tag="w",
                                          name=f"wo_stage{ko}")
                    nc.sync.dma_start(
                        out=stage, in_=f32r(wo_d[128 * ko:128 * (ko + 1), :]))
                    stages.append(stage)
                for ko in range(8):
                    w_sb = wo_pool.tile([PPART, 1024], BF, tag="w",
                                        name=f"wo_bf{ko}")
                    nc.gpsimd.tensor_copy(out=w_sb, in_=stages[ko])
                    wo_tiles[ko] = w_sb

            # --------------- attention (pipelined) ---------------
            # per head: 3 PSUM banks -- kt0 | kt1+kt3 (shared) | kt2 --
            # and 3 exp ops; S(i+1) is emitted before AV(i) and `filler`
            # supplies extra PE work that hides the exp chain latency.
            state = {}

            def attn_S(b, h, q_t, k_t, sps, tag="s", bufs=3):
                base = 64 * (h % 2)
                j = h // 2
                ps0 = sps.tile([PPART, T], DT, tag=tag, bufs=bufs, name="ps0")
                ps13 = sps.tile([PPART, T], DT, tag=tag, bufs=bufs, name="ps13")
                ps2 = sps.tile([PPART, T], DT, tag=tag, bufs=bufs, name="ps2")
                place = {0: (ps0, 0, 512), 1: (ps13, 0, 384),
                         2: (ps2, 0, 256), 3: (ps13, 384, 128)}
                for kt in range(4):
                    ps, off, nq = place[kt]
                    nc.tensor.matmul(
                        ps[:, off:off + nq],
                        lhsT=k_t[base:base + 64, j, 128 * kt:128 * (kt + 1)],
                        rhs=q_t[base:base + 64, j, 128 * kt:],
                        start=True, stop=True)
                pch0 = p_pool.tile([PPART, T], BF, tag="P")
                pch13 = p_pool.tile([PPART, T], BF, tag="P")
                pch2 = p_pool.tile([PPART, T], BF, tag="P")
                nc.scalar.activation(out=pch0, in_=ps0,
                                     func=ACTF.Exp, scale=1.0 / 32.0)
                nc.scalar.activation(out=pch13, in_=ps13,
                                     func=ACTF.Exp, scale=1.0 / 32.0)
                nc.scalar.activation(out=pch2[:, :256], in_=ps2[:, :256],
                                     func=ACTF.Exp, scale=1.0 / 32.0)
                # zero the masked (k > q) parts of the diagonal blocks
                for pc, off in ((pch0, 0), (pch13, 0), (pch2, 0),
                                (pch13, 384)):
                    nc.vector.tensor_mul(
                        out=pc[:, off:off + 128], in0=pc[:, off:off + 128],
                        in1=mask1_b)
                state[(b, h)] = (pch0, pch13, pch2)

            def attn_AV(b, h, v_t, yps):
                base = 64 * (h % 2)
                j = h // 2
                pch0, pch13, pch2 = state.pop((b, h))
                rhs = {0: pch0[:, :512], 1: pch13[:, :384],
                       2: pch2[:, :256], 3: pch13[:, 384:512]}
                psy = yps.tile([VW, T], DT, tag="y")
                for kt in range(4):
                    nc.tensor.matmul(
                        psy[:, 128 * kt:],
                        lhsT=v_t[:, kt, VW * h:VW * (h + 1)],
                        rhs=rhs[kt],
                        start=(kt == 0), stop=(kt == 3))
                r_row = s_pool.tile([1, T], BF, tag="r1", bufs=2)
                with nc.allow_low_precision(reason="bf16 softmax recip row"):
                    nc.vector.reciprocal(r_row, psy[HD:HD + 1, :])
                r64 = s_pool.tile([HD, T], BF, tag="r64")
                nc.gpsimd.partition_broadcast(r64, r_row, channels=HD)
                state[(b, h, "n")] = (psy, r64)

            def attn_norm(b, h):
                base = 64 * (h % 2)
                j = h // 2
                psy, r64 = state.pop((b, h, "n"))
                nc.vector.tensor_mul(
                    out=y_t[base:base + 64, j, T * b:T * (b + 1)],
                    in0=psy[:HD, :], in1=r64)

            def wo_span(ci, ko_lo, ko_hi, sps, out_f):
                # ko_lo..ko_hi accumulations of w_o chunk ci (+evac at end)
                tg, nh = ci // 2, ci % 2
                if ko_lo == 0:
                    state[("wo", ci)] = sps.tile([PPART, T], DT, tag="wops",
                                                 bufs=2, name="wops")
                ps = state[("wo", ci)]
                for ko in range(ko_lo, ko_hi + 1):
                    nc.tensor.matmul(
                        ps,
                        lhsT=y_t[:, ko, 128 * tg:128 * (tg + 1)],
                        rhs=wo_tiles[ko][:, 512 * nh:512 * (nh + 1)],
                        start=(ko == 0), stop=(ko == 7))
                if ko_hi == 7:
                    state.pop(("wo", ci))
                    ob = o_pool.tile([PPART, T], DT)
                    nc.vector.tensor_add(
                        out=ob, in0=ps, in1=bo_bc[:, 512 * nh:512 * (nh + 1)])
                    nc.sync.dma_start(
                        out=out_f[128 * tg:128 * (tg + 1),
                                  512 * nh:512 * (nh + 1)], in_=ob)

            def wo_chunk(tg, nh, sps, out_f):
                ps = sps.tile([PPART, T], DT, tag="wops", bufs=2)
                for ko in range(8):
                    nc.tensor.matmul(
                        ps,
                        lhsT=y_t[:, ko, 128 * tg:128 * (tg + 1)],
                        rhs=wo_tiles[ko][:, 512 * nh:512 * (nh + 1)],
                        start=(ko == 0), stop=(ko == 7))
                ob = o_pool.tile([PPART, T], DT)
                nc.vector.tensor_add(
                    out=ob, in0=ps, in1=bo_bc[:, 512 * nh:512 * (nh + 1)])
                nc.sync.dma_start(
                    out=out_f[128 * tg:128 * (tg + 1),
                              512 * nh:512 * (nh + 1)], in_=ob)

            def attn_phase(b, qkv, sps, yps, filler, tail1, tail2,
                           pre_s=0):
                q_t, k_t, v_t = qkv["q"], qkv["k"], qkv["v"]
                if pre_s == 0:
                    attn_S(b, 0, q_t, k_t, sps)
                for i in range(H):
                    filler(i)
                    if max(i, pre_s - 1) < i + 1 < H:
                        attn_S(b, i + 1, q_t, k_t, sps)
                    attn_AV(b, i, v_t, yps)
                    if i >= 2:
                        attn_norm(b, i - 2)
                tail1()
                attn_norm(b, H - 2)
                attn_norm(b, H - 1)
                tail2()

            # ---------------- schedule ----------------
            for rep in range(reps):
              out_f = out_fs[rep]
              sfx = str(rep)
              # startup: interleave x(b0) and w_qkv sec0 DMAs so neither
              # serializes behind the other; sec0 stays resident for a
              # ko-outer Q(b0) that streams sec1 behind it.
              with tc.tile_pool(name="ps0" + sfx, bufs=2, space="PSUM") as trps:
                x_t = xt_pool.tile([PPART, 8, T], R)
                xns0 = [start_xn_dma(0, 0, split=True)]
                for to in range(1, 4):
                    xns0.append(start_xn_dma(0, to))
                s0 = [w_dma(wqkv_d[128 * ko:128 * (ko + 1), 0:1024])
                      for ko in range(8)]
                if rep == 0:
                    load_biases()
                for to in range(4):
                    for fg in range(2):
                        transpose_fg(x_t, xns0[to], to, fg, trps)
              with tc.tile_pool(name="qps0" + sfx, bufs=8, space="PSUM") as qps:
                qkv0 = alloc_qkv("qkv")
                qkv_sec_stream(x_t, qps, qkv0["q"], 0, tiles=s0)
                qkv_sec_stream(x_t, qps, qkv0["k"], 1)
                qkv_v_sec(0, x_t, qps, qkv0["v"], resident=True)
                attn_S(0, 0, qkv0["q"], qkv0["k"], qps, tag="ps", bufs=8)
              with (
                tc.tile_pool(name="aps0" + sfx, bufs=3, space="PSUM") as sps,
                tc.tile_pool(name="atr0" + sfx, bufs=2, space="PSUM") as trps,
                tc.tile_pool(name="ay0" + sfx, bufs=3, space="PSUM") as yps,
              ):
                x_t2 = xt_pool.tile([PPART, 8, T], R)
                xns = [start_xn_dma(1, to) for to in range(4)]
                pre = preload_sec(0)
                k1_tiles = [w_dma(wqkv_d[128 * ko:128 * (ko + 1), 1024:2048])
                            for ko in range(8)]
                q1 = alloc_qkv("q")

                def q_fo(fo):
                    ps = trps.tile([PPART, 4 * PPART], DT, tag="tr")
                    for ko in range(8):
                        nc.tensor.matmul(
                            ps,
                            lhsT=pre[ko][:, 128 * fo:128 * (fo + 1)],
                            rhs=x_t2[:, ko, :],
                            start=(ko == 0), stop=(ko == 7))
                    q_evac(ps, q1["q"], fo)

                def filler_b0(i):
                    if i < 8:   # x^T(b1): one transpose group per head
                        transpose_fg(x_t2, xns[i // 2], i // 2, i % 2, trps)
                    elif i < 15:  # Q(b1) fo-group per head from preloaded w
                        q_fo(i - 8)

                def tail1_b0():
                    q_fo(7)
                    load_wo()

                attn_phase(0, qkv0, sps, yps, filler_b0,
                           tail1_b0, lambda: None, pre_s=1)
                x_t = x_t2
              with tc.tile_pool(name="qps1" + sfx, bufs=8, space="PSUM") as qps:
                kv1 = alloc_qkv("kv")
                qkv_sec_stream(x_t, qps, kv1["k"], 1, tiles=k1_tiles)
                qkv_v_sec(1, x_t, qps, kv1["v"], resident=True)
                qkv1 = {"q": q1["q"], "k": kv1["k"], "v": kv1["v"]}
                attn_S(1, 0, qkv1["q"], qkv1["k"], qps, tag="ps", bufs=8)
              with (
                tc.tile_pool(name="aps1" + sfx, bufs=3, space="PSUM") as sps,
                tc.tile_pool(name="ay1" + sfx, bufs=3, space="PSUM") as yps,
              ):
                def filler_b1(i):
                    wo_span(i // 2, 4 * (i % 2), 4 * (i % 2) + 3, sps, out_f)

                def tail1_b1():
                    # ko 0-6 (heads 0-13, already normalized) of the first
                    # two tail chunks cover the final normalize latency
                    wo_span(8, 0, 6, sps, out_f)
                    wo_span(9, 0, 6, sps, out_f)

                def tail2_b1():
                    wo_span(8, 7, 7, sps, out_f)
                    wo_span(9, 7, 7, sps, out_f)
                    for tg in range(5, 8):
                        for nh in range(2):
                            wo_chunk(tg, nh, sps, out_f)

                attn_phase(1, qkv1, sps, yps, filler_b1, tail1_b1,
                           tail2_b1, pre_s=1)

    nc.compile()
    return nc


def _get_program(reps=1, phases="xqaw"):
    key = f"nc{reps}{phases}"
    if key not in _CACHE:
        _CACHE[key] = _build_program(reps, phases)
    return _CACHE[key]


def kernel(x, w_qkv, b_qkv, w_o, b_o):
    from concourse.bass_utils import run_bass_kernel_spmd

    nc = _get_program()
    x = np.ascontiguousarray(x, dtype=np.float32)
    in_maps = []
    for c in range(NCORES):
        in_maps.append({
            "x": x[BL * c:BL * (c + 1)],
            "w_qkv": np.asarray(w_qkv, dtype=np.float32),
            "b_qkv": np.asarray(b_qkv, dtype=np.float32),
            "w_o": np.asarray(w_o, dtype=np.float32),
            "b_o": np.asarray(b_o, dtype=np.float32),
        })
    res = run_bass_kernel_spmd(nc, in_maps, list(range(NCORES)))
    return np.concatenate([res.results[c]["out"] for c in range(NCORES)], axis=0)
